# revision 42
# baseline (speedup 1.0000x reference)
"""Bass/Trainium2 kernel for nn_EF_42511586295882 (GNN message passing).

Math reduction proven against reference: only the l=0 spherical channel of
iteration 0 reaches the output (refinement mixes features, never l-channels,
and only x[:, 0, :] feeds iteration 1 / readout).  The whole computation is:

  rad[e,k]  = T_k(2*exp(-r)-1) * cut(r)                        (E,16)
  msg0[e,f] = (rad @ (0.282095*Wr1_0 + Wr2_0))[e,f] * embed[z[src_e], f]
  X0[a,f]   = sum_{e: dst=a} msg0[e,f]
  x0        = X0 + (h0 * silu(h0)) @ W2_0,   h0 = X0 @ W1_0
  msg1[e,f] = (rad @ Wr1_1)[e,f] * x0[src_e, f]
  X1[a,f]   = sum_{e: dst=a} msg1[e,f]
  x0b       = X1 + silu(X1 @ W1_1) @ W2_1
  e_atom    = x0b @ w_out + b_out[z] + zbl_pair_sum[a]
  e_mol     = segment_sum(e_atom * atom_mask, batch_segments)

This implementation is optimized for END-TO-END wall time of kernel():
host numpy prep + PJRT/axon transfer + device execution.  Key choices:

  * ONE device launch (no host round-trip between the two message passes):
    x0 is exchanged on-device with an 8-core DRAM AllGather, and x0[src] /
    embed[z_src] are fetched per 128-edge tile with indirect (indexed) DMA.
  * Only 9B/edge is shipped (r f32, dst-slot u8, z_src u8, z_dst u8,
    src-row u16); all O(E) and O(E*F) math, including the ZBL pair energy,
    happens on device.  Host prep is ~20ms of numpy.
  * Edges only need to be GROUPED by 128-atom destination block, not fully
    sorted: argsort on a uint8 block key is ~20x faster than on dst.
  * The jax.jit(shard_map(...)) dispatcher is built once and cached
    (rebuilding it per call costs ~0.15s extra), and the edge planes are
    device_put asynchronously as soon as each is computed so transfers
    overlap the remaining host prep.  The per-launch round-trip floor of
    this axon stack is ~75ms; everything else is hidden behind it.

Sharding: core k owns atoms [2048k, 2048(k+1)) and all edges into them,
grouped in 16 aligned 128-atom blocks.  Scatter = one-hot matmul into a
per-block PSUM accumulator.
"""

import math
import numpy as np

P = 128
N = 16384
E = 262144
B = 512
F = 32
K = 16
NZ = 119
NCORES = 8
AC = N // NCORES          # atoms per core
NB = AC // P              # 128-atom blocks per core (16)
CUTOFF = 6.0
KE = 14.399645
ZBL_C = [0.18175, 0.50986, 0.28022, 0.02817]
ZBL_D = [3.19980, 0.94229, 0.40290, 0.20162]
A_PRE = 0.8854 * 0.529177

_CACHE = {}
_SHARDING = None


def _get_sharding():
    """Mesh/sharding is fixed (8 cores, rows split) independent of T, so
    transfers can start before the edge layout — and thus the runner — is
    known."""
    global _SHARDING
    if _SHARDING is None:
        import jax
        from jax.sharding import Mesh, PartitionSpec, NamedSharding
        mesh = Mesh(np.asarray(jax.devices()[:NCORES]), ("core",))
        _SHARDING = NamedSharding(mesh, PartitionSpec("core"))
    return _SHARDING


# --------------------------------------------------------------------------
# host prep: group edges by dst 128-block, lay them out in (core, 128-slot,
# tile) grid, compute r + ZBL pair energies, emit [NCORES*P, ...] arrays.
# Split in two stages so the big edge planes can start transferring to the
# devices (async device_put) while stage 2 runs.
# --------------------------------------------------------------------------
_ARANGE_E = None


_SROW_LUT = None


def _prep_stage1(positions, dst_idx, src_idx, an, stage_cb):
    """Build the per-edge planes.  ``stage_cb(T, T_blk, name, arr)`` is
    called as soon as each plane is final so its device transfer can start
    while the remaining planes are still being computed.  The r chain runs
    on the main thread concurrently with the integer blob in a worker."""
    global _ARANGE_E, _SROW_LUT
    import concurrent.futures as _cf
    pos = np.asarray(positions, dtype=np.float32)
    dst = np.asarray(dst_idx).astype(np.int32)
    src = np.asarray(src_idx).astype(np.int32)

    # group by destination 128-atom block (order within a block is free)
    order = np.argsort((dst >> 7).astype(np.uint8), kind="stable")
    dsts = np.take(dst, order)
    srcs = np.take(src, order)

    gb = dsts >> 7                       # global 128-atom block id (0..127)
    cnt = np.bincount(gb, minlength=N // P)
    T_blk = int(math.ceil(cnt.max() / P))
    T = NB * T_blk

    starts = np.zeros(N // P, dtype=np.int32)
    np.cumsum(cnt[:-1], out=starts[1:], dtype=np.int32)
    if _ARANGE_E is None or len(_ARANGE_E) != len(dsts):
        _ARANGE_E = np.arange(len(dsts), dtype=np.int32)
    k = _ARANGE_E - np.take(starts, gb)
    # flat position in concat [NCORES*P, T] via two small LUTs:
    # flat = (gb>>4)*(P*T) + (gb&15)*T_blk  +  (k&127)*T + (k>>7)
    lut_g = ((np.arange(N // P, dtype=np.int32) >> 4) * (P * T)
             + (np.arange(N // P, dtype=np.int32) & 15) * T_blk)
    kmax = int(cnt.max())
    lut_k = ((np.arange(kmax, dtype=np.int32) & 127) * T
             + (np.arange(kmax, dtype=np.int32) >> 7))
    flat = np.take(lut_g, gb)
    flat += np.take(lut_k, k)

    NPALL = NCORES * P

    def int_planes():
        # one u8 blob [NPALL, 5T]: dloc | zsrc | zdst | srow(u16)
        blob = np.zeros((NPALL, 5 * T), dtype=np.uint8)
        dlo = np.zeros((NPALL * T,), dtype=np.uint8)
        dlo[flat] = (dsts & 127).astype(np.uint8)
        blob[:, 0:T] = dlo.reshape(NPALL, T)
        an8 = an.astype(np.uint8)
        zsr = np.zeros((NPALL * T,), dtype=np.uint8)
        zsr[flat] = np.take(an8, srcs)
        blob[:, T:2 * T] = zsr.reshape(NPALL, T)
        zds = np.zeros((NPALL * T,), dtype=np.uint8)
        zds[flat] = np.take(an8, dsts)
        blob[:, 2 * T:3 * T] = zds.reshape(NPALL, T)
        # srow: row of atom a in the allgathered x0 table [N, F]:
        # R(a) = (a//2048)*2048 + (a%128)*16 + (a%2048)//128  (LUT over a)
        global _SROW_LUT
        if _SROW_LUT is None:
            a_all = np.arange(N, dtype=np.int32)
            _SROW_LUT = (((a_all >> 11) << 11) + ((a_all & 127) << 4)
                         + ((a_all & 2047) >> 7)).astype(np.uint16)
        srow = np.zeros((NPALL * T,), dtype=np.uint16)
        srow[flat] = np.take(_SROW_LUT, srcs)
        blob[:, 3 * T:5 * T] = srow.view(np.uint8).reshape(NPALL, 2 * T)
        return blob

    with _cf.ThreadPoolExecutor(max_workers=1) as ex:
        bi_fut = ex.submit(int_planes)

        # per-edge r via contiguous per-column ops (main thread)
        px, py, pz = pos[:, 0].copy(), pos[:, 1].copy(), pos[:, 2].copy()
        dx = np.take(px, srcs)
        dx -= np.take(px, dsts)
        dy = np.take(py, srcs)
        dy -= np.take(py, dsts)
        dz = np.take(pz, srcs)
        dz -= np.take(pz, dsts)
        np.multiply(dx, dx, out=dx)
        np.multiply(dy, dy, out=dy)
        np.multiply(dz, dz, out=dz)
        dx += dy
        dx += dz
        dx += 1e-10
        r = np.sqrt(dx, out=dx)
        np.maximum(r, 1e-4, out=r)
        # f16 r: rel err ~5e-4 on r -> 1.7e-4 on e_mol (gate 2e-2), halves the plane
        rpl = np.full((NPALL * T,), 1000.0, dtype=np.float16)  # padded: cut=0
        rpl[flat] = r.astype(np.float16)

        blob = bi_fut.result()
    stage_cb(T, T_blk, "blob_i", blob)
    stage_cb(T, T_blk, "r_t", rpl.reshape(NPALL, T))

    return T, T_blk


def _prep_stage2(an, batch_segments, atom_mask, embed,
                 Wr1_0, Wr2_0, W1_0, W2_0, Wr1_1, W1_1, W2_1, w_out, b_out):
    seg = np.asarray(batch_segments).astype(np.int64)

    # ---- per-atom planes [NCORES*P, NB]: atom a=(c,b,p) -> row c*128+p, col b
    def atom_plane(v):
        return v.reshape(NCORES, NB, P).transpose(0, 2, 1)

    mol_base = seg.reshape(NCORES, AC)[:, 0]
    segloc = (seg - np.repeat(mol_base, AC)).astype(np.float32)
    assert segloc.max() < P, "molecule window exceeds 128 per core"

    # iota | w_out | b_out[z] | segloc | atom_mask in one [P, 208] array
    ioa = np.empty((NCORES, P, P + F + 3 * NB), dtype=np.float32)
    ioa[:, :, 0:P] = np.arange(P, dtype=np.float32)[None, None, :]
    ioa[:, :, P:P + F] = np.asarray(w_out, np.float32)[None, None, :]
    o = P + F
    ioa[:, :, o:o + NB] = atom_plane(np.take(np.asarray(b_out, np.float32), an))
    ioa[:, :, o + NB:o + 2 * NB] = atom_plane(segloc)
    ioa[:, :, o + 2 * NB:o + 3 * NB] = atom_plane(np.asarray(atom_mask, np.float32))

    # ---- replicated small tensors ----
    gcW = 0.282095 * np.asarray(Wr1_0, np.float32) + np.asarray(Wr2_0, np.float32)
    wcatS = np.empty((K, 2 * F), dtype=np.float32)
    wcatS[:, 0:F] = gcW
    wcatS[:, F:2 * F] = np.asarray(Wr1_1, np.float32)

    wpack = np.empty((F, 4 * F), dtype=np.float32)
    wpack[:, 0 * F:1 * F] = np.asarray(W1_0, np.float32)
    wpack[:, 1 * F:2 * F] = np.asarray(W2_0, np.float32)
    wpack[:, 2 * F:3 * F] = np.asarray(W1_1, np.float32)
    wpack[:, 3 * F:4 * F] = np.asarray(W2_1, np.float32)

    embP = np.zeros((P, F), dtype=np.float32)
    embP[:NZ] = np.asarray(embed, dtype=np.float32)

    def rep(a):            # replicate a per-core tensor 8x along axis 0
        return np.ascontiguousarray(
            np.broadcast_to(a[None], (NCORES, *a.shape))).reshape(
                NCORES * a.shape[0], *a.shape[1:])

    arrays = {
        "ioa": ioa.reshape(NCORES * P, P + F + 3 * NB),
        "wcatS": rep(wcatS),
        "wpack": rep(wpack),
        "embP": rep(embP),
    }
    return arrays, mol_base


# --------------------------------------------------------------------------
# device kernel: one NEFF doing pass1 -> refine0 -> AllGather -> pass2 ->
# refine1 -> readout
# --------------------------------------------------------------------------
def _build(T, T_blk):
    import concourse.bacc as bacc
    import concourse.bass as bass
    import concourse.mybir as mybir
    import concourse.tile as tile
    from concourse.masks import make_identity

    f32 = mybir.dt.float32
    f16 = mybir.dt.float16
    i32 = mybir.dt.int32
    u8 = mybir.dt.uint8
    u16 = mybir.dt.uint16
    ALU = mybir.AluOpType
    ACT = mybir.ActivationFunctionType

    nc = bacc.Bacc("TRN2", target_bir_lowering=False, debug=False,
                   num_devices=NCORES)

    d_r = nc.dram_tensor("r_t", [P, T], f16, kind="ExternalInput")
    d_bi = nc.dram_tensor("blob_i", [P, 5 * T], u8, kind="ExternalInput")
    d_wcatS = nc.dram_tensor("wcatS", [K, 2 * F], f32, kind="ExternalInput")
    d_wpack = nc.dram_tensor("wpack", [F, 4 * F], f32, kind="ExternalInput")
    d_ioa = nc.dram_tensor("ioa", [P, P + F + 3 * NB], f32,
                           kind="ExternalInput")
    d_embP = nc.dram_tensor("embP", [P, F], f32, kind="ExternalInput")
    d_out = nc.dram_tensor("out", [P, 1], f32, kind="ExternalOutput")

    with tile.TileContext(nc) as tc:
        with tc.tile_pool(name="const", bufs=1) as cpool, \
             tc.tile_pool(name="persist", bufs=1) as pp, \
             tc.tile_pool(name="dram", bufs=1, space="DRAM") as dpool:

            ident = cpool.tile([P, P], f32, tag="ident")
            make_identity(nc, ident[:])
            ioa = cpool.tile([P, P + F + 3 * NB], f32, tag="ioa")
            nc.sync.dma_start(ioa[:], d_ioa[:, :])
            iota = ioa[:, 0:P]
            woutr = ioa[:, P:P + F]
            _o = P + F
            bout_t = ioa[:, _o:_o + NB]
            segloc_t = ioa[:, _o + NB:_o + 2 * NB]
            amask_t = ioa[:, _o + 2 * NB:_o + 3 * NB]
            wcat = cpool.tile([P, 2 * F], f32, tag="wcat")
            for g in range(4):
                nc.sync.dma_start(wcat[32 * g:32 * g + K, :], d_wcatS[:, :])
            wpack = cpool.tile([F, 4 * F], f32, tag="wpack")
            nc.sync.dma_start(wpack[:], d_wpack[:, :])
            w10 = wpack[:, 0 * F:1 * F]
            w20 = wpack[:, 1 * F:2 * F]
            w11 = wpack[:, 2 * F:3 * F]
            w21 = wpack[:, 3 * F:4 * F]

            dloc8 = pp.tile([P, T], u8, tag="dloc8")
            nc.sync.dma_start(dloc8[:], d_bi[:, 0:T])
            dloc = pp.tile([P, T], f32, tag="dloc")
            nc.vector.tensor_copy(out=dloc[:], in_=dloc8[:])
            zsrc8 = pp.tile([P, T], u8, tag="zsrc8")
            nc.sync.dma_start(zsrc8[:], d_bi[:, T:2 * T])
            zsrc = pp.tile([P, T], i32, tag="zsrc")
            nc.vector.tensor_copy(out=zsrc[:], in_=zsrc8[:])
            srow16 = pp.tile([P, T], u16, tag="srow16")
            nc.sync.dma_start(srow16[:], d_bi[:, 3 * T:5 * T].bitcast(u16))
            srow = pp.tile([P, T], i32, tag="srow")
            nc.vector.tensor_copy(out=srow[:], in_=srow16[:])

            g_all = pp.tile([P, T, F], f32, tag="g_all")
            epair = pp.tile([P, T], f32, tag="epair")
            X0sb = pp.tile([P, NB, F], f32, tag="X0sb")
            x0sb = pp.tile([P, NB, F], f32, tag="x0sb")

            in_b = dpool.tile([P, NB * F], f32)
            x0tab = dpool.tile([N, F], f32)

            # ---------------- pass 1: edge math + scatter ----------------
            with tc.tile_pool(name="p1", bufs=1) as p1, \
                 tc.tile_pool(name="rot", bufs=4) as rot, \
                 tc.tile_pool(name="ps_rt", bufs=2, space="PSUM") as ps_rt, \
                 tc.tile_pool(name="ps_g", bufs=2, space="PSUM") as ps_g, \
                 tc.tile_pool(name="ps_x", bufs=2, space="PSUM") as ps_x:

                r16 = p1.tile([P, T], f16, tag="r16")
                nc.sync.dma_start(r16[:], d_r[:, :])
                r = p1.tile([P, T], f32, tag="r")
                nc.vector.tensor_copy(out=r[:], in_=r16[:])

                # t = 2*exp(-r) - 1 ; t2 = 2*t
                tch = p1.tile([P, T], f32, tag="tch")
                nc.scalar.activation(out=tch[:], in_=r[:], func=ACT.Exp,
                                     scale=-1.0)
                t2 = p1.tile([P, T], f32, tag="t2")
                nc.vector.tensor_scalar(out=t2[:], in0=tch[:], scalar1=4.0,
                                        scalar2=-2.0, op0=ALU.mult, op1=ALU.add)
                nc.vector.tensor_scalar(out=tch[:], in0=tch[:], scalar1=2.0,
                                        scalar2=-1.0, op0=ALU.mult, op1=ALU.add)

                # cut = exp(-u2/(1-u2)), u = min(r/C, 1-1e-6)
                u = p1.tile([P, T], f32, tag="u")
                nc.vector.tensor_scalar(out=u[:], in0=r[:],
                                        scalar1=1.0 / CUTOFF,
                                        scalar2=1.0 - 1e-6,
                                        op0=ALU.mult, op1=ALU.min)
                u2 = p1.tile([P, T], f32, tag="u2")
                nc.vector.tensor_tensor(out=u2[:], in0=u[:], in1=u[:],
                                        op=ALU.mult)
                den = p1.tile([P, T], f32, tag="den")
                nc.vector.tensor_scalar(out=den[:], in0=u2[:], scalar1=-1.0,
                                        scalar2=1.0, op0=ALU.mult, op1=ALU.add)
                nc.vector.reciprocal(out=den[:], in_=den[:])
                frac = p1.tile([P, T], f32, tag="frac")
                nc.vector.tensor_tensor(out=frac[:], in0=u2[:], in1=den[:],
                                        op=ALU.mult)
                cutm = p1.tile([P, T], f32, tag="cutm")
                nc.scalar.activation(out=cutm[:], in_=frac[:], func=ACT.Exp,
                                     scale=-1.0)

                # Chebyshev ladder seeded with cut: rad_k = T_k(t)*cut
                rad = p1.tile([P, T, 2 * K], f32, tag="rad")
                nc.vector.memset(rad[:], 0.0)
                nc.vector.tensor_copy(out=rad[:, :, 0], in_=cutm[:])
                nc.vector.tensor_tensor(out=rad[:, :, 1], in0=tch[:],
                                        in1=cutm[:], op=ALU.mult)
                tmp = p1.tile([P, T], f32, tag="tmp")
                for kk in range(2, K):
                    nc.vector.tensor_tensor(out=tmp[:], in0=t2[:],
                                            in1=rad[:, :, kk - 1], op=ALU.mult)
                    nc.vector.tensor_tensor(out=rad[:, :, kk], in0=tmp[:],
                                            in1=rad[:, :, kk - 2],
                                            op=ALU.subtract)

                # ---- ZBL pair energy (scattered via pass-2 extra column) --
                zdst8 = p1.tile([P, T], u8, tag="zdst8")
                nc.sync.dma_start(zdst8[:], d_bi[:, 2 * T:3 * T])
                zdf = p1.tile([P, T], f32, tag="zdf")
                nc.vector.tensor_copy(out=zdf[:], in_=zdst8[:])
                zsf = p1.tile([P, T], f32, tag="zsf")
                nc.vector.tensor_copy(out=zsf[:], in_=zsrc8[:])
                zz = p1.tile([P, T], f32, tag="zz")
                nc.vector.tensor_tensor(out=zz[:], in0=zdf[:], in1=zsf[:],
                                        op=ALU.mult)
                # zpow = exp(0.23*ln(max(z,1))); z=0 edges have zz=0 anyway
                lnz = p1.tile([P, T], f32, tag="lnz")
                zpd = p1.tile([P, T], f32, tag="zpd")
                nc.vector.tensor_scalar_max(out=zpd[:], in0=zdf[:], scalar1=1.0)
                nc.scalar.activation(out=lnz[:], in_=zpd[:], func=ACT.Ln)
                nc.scalar.activation(out=zpd[:], in_=lnz[:], func=ACT.Exp,
                                     scale=0.23)
                zps = p1.tile([P, T], f32, tag="zps")
                nc.vector.tensor_scalar_max(out=zps[:], in0=zsf[:], scalar1=1.0)
                nc.scalar.activation(out=lnz[:], in_=zps[:], func=ACT.Ln)
                nc.scalar.activation(out=zps[:], in_=lnz[:], func=ACT.Exp,
                                     scale=0.23)
                ra = p1.tile([P, T], f32, tag="ra")
                nc.vector.tensor_tensor(out=ra[:], in0=zpd[:], in1=zps[:],
                                        op=ALU.add)
                nc.vector.tensor_tensor(out=ra[:], in0=ra[:], in1=r[:],
                                        op=ALU.mult)
                nc.vector.tensor_scalar_mul(out=ra[:], in0=ra[:],
                                            scalar1=1.0 / A_PRE)
                phi = p1.tile([P, T], f32, tag="phi")
                ej = p1.tile([P, T], f32, tag="ej")
                for jj in range(4):
                    nc.scalar.activation(out=ej[:], in_=ra[:], func=ACT.Exp,
                                         scale=-ZBL_D[jj])
                    if jj == 0:
                        nc.vector.tensor_scalar_mul(out=phi[:], in0=ej[:],
                                                    scalar1=ZBL_C[jj])
                    else:
                        nc.vector.tensor_scalar_mul(out=ej[:], in0=ej[:],
                                                    scalar1=ZBL_C[jj])
                        nc.vector.tensor_tensor(out=phi[:], in0=phi[:],
                                                in1=ej[:], op=ALU.add)
                rinv = p1.tile([P, T], f32, tag="rinv")
                nc.vector.reciprocal(out=rinv[:], in_=r[:])
                nc.vector.tensor_tensor(out=epair[:], in0=zz[:], in1=phi[:],
                                        op=ALU.mult)
                nc.vector.tensor_tensor(out=epair[:], in0=epair[:],
                                        in1=rinv[:], op=ALU.mult)
                nc.vector.tensor_tensor(out=epair[:], in0=epair[:],
                                        in1=cutm[:], op=ALU.mult)
                nc.vector.tensor_scalar_mul(out=epair[:], in0=epair[:],
                                            scalar1=0.5 * KE)

                for b in range(NB):
                    x0ps = ps_x.tile([P, F], f32, tag="x0ps")
                    for j in range(T_blk):
                        t = b * T_blk + j
                        g4 = t % 4
                        if g4 == 0:
                            radT = ps_rt.tile([P, P], f32, tag="radT")
                            hi = min(4, T - t)
                            nc.tensor.transpose(
                                out=radT[0:32 * hi, :],
                                in_=rad[:, t:t + hi, :],
                                identity=ident[:])
                            radTs = rot.tile([P, P], f32, tag="radTs")
                            nc.scalar.copy(out=radTs[0:32 * hi, :],
                                           in_=radT[0:32 * hi, :])
                        gps = ps_g.tile([P, 2 * F], f32, tag="gps")
                        nc.tensor.matmul(out=gps[:],
                                         lhsT=radTs[32 * g4:32 * g4 + 32, :],
                                         rhs=wcat[32 * g4:32 * g4 + 32, :],
                                         start=True, stop=True,
                                         tile_position=(32 * g4, 0))
                        xs0 = rot.tile([P, F], f32, tag="xs0")
                        nc.gpsimd.indirect_dma_start(
                            out=xs0[:], out_offset=None,
                            in_=d_embP[:, :],
                            in_offset=bass.IndirectOffsetOnAxis(
                                ap=zsrc[:, t:t + 1], axis=0))
                        oh = rot.tile([P, P], f32, tag="oh")
                        nc.vector.tensor_scalar(out=oh[:], in0=iota,
                                                scalar1=dloc[:, t:t + 1],
                                                scalar2=None, op0=ALU.is_equal)
                        msg = rot.tile([P, F], f32, tag="msg")
                        nc.vector.tensor_tensor(out=msg[:], in0=gps[:, 0:F],
                                                in1=xs0[:], op=ALU.mult)
                        nc.scalar.copy(out=g_all[:, t, :], in_=gps[:, F:2 * F])
                        nc.tensor.matmul(out=x0ps[:], lhsT=oh[:], rhs=msg[:],
                                         start=(j == 0), stop=(j == T_blk - 1))
                    nc.scalar.copy(out=X0sb[:, b, :], in_=x0ps[:])

            # ---------------- refinement 0 ----------------
            with tc.tile_pool(name="rf", bufs=2) as rf, \
                 tc.tile_pool(name="rps1", bufs=2, space="PSUM") as rps1, \
                 tc.tile_pool(name="rps2", bufs=2, space="PSUM") as rps2:
                for b in range(NB):
                    trp = rps1.tile([F, P], f32, tag="trp")
                    nc.tensor.transpose(out=trp[:], in_=X0sb[:, b, :],
                                        identity=ident[:])
                    xT = rf.tile([F, P], f32, tag="xT")
                    nc.scalar.copy(out=xT[:], in_=trp[:])
                    hps = rps2.tile([P, F], f32, tag="hps")
                    nc.tensor.matmul(out=hps[:], lhsT=xT[:], rhs=w10,
                                     start=True, stop=True)
                    sw = rf.tile([P, F], f32, tag="sw")
                    nc.scalar.activation(out=sw[:], in_=hps[:], func=ACT.Silu)
                    gate = rf.tile([P, F], f32, tag="gate")
                    nc.vector.tensor_tensor(out=gate[:], in0=hps[:], in1=sw[:],
                                            op=ALU.mult)
                    gtp = rps1.tile([F, P], f32, tag="trp")
                    nc.tensor.transpose(out=gtp[:], in_=gate[:],
                                        identity=ident[:])
                    gT = rf.tile([F, P], f32, tag="gT")
                    nc.scalar.copy(out=gT[:], in_=gtp[:])
                    dps = rps2.tile([P, F], f32, tag="hps")
                    nc.tensor.matmul(out=dps[:], lhsT=gT[:], rhs=w20,
                                     start=True, stop=True)
                    nc.vector.tensor_tensor(out=x0sb[:, b, :],
                                            in0=X0sb[:, b, :], in1=dps[:],
                                            op=ALU.add)

            # ---------------- exchange: AllGather x0 ----------------
            nc.sync.dma_start(in_b[:], x0sb[:])
            nc.gpsimd.collective_compute(
                "AllGather", ALU.bypass,
                replica_groups=[list(range(NCORES))],
                ins=[in_b.opt()], outs=[x0tab.opt()])

            # ---------------- pass 2 + refinement 1 + readout -------------
            with tc.tile_pool(name="p2", bufs=1) as p2, \
                 tc.tile_pool(name="rot2", bufs=4) as rot2, \
                 tc.tile_pool(name="rf2", bufs=2) as rf2, \
                 tc.tile_pool(name="p2ps", bufs=2, space="PSUM") as p2ps, \
                 tc.tile_pool(name="rps1", bufs=2, space="PSUM") as rps1, \
                 tc.tile_pool(name="rps2", bufs=2, space="PSUM") as rps2, \
                 tc.tile_pool(name="psm", bufs=1, space="PSUM") as psm:

                X1sb = p2.tile([P, NB, F + 1], f32, tag="X1sb")
                for b in range(NB):
                    x1ps = p2ps.tile([P, F + 1], f32, tag="x1ps")
                    for j in range(T_blk):
                        t = b * T_blk + j
                        xg = rot2.tile([P, F], f32, tag="xg")
                        nc.gpsimd.indirect_dma_start(
                            out=xg[:], out_offset=None,
                            in_=x0tab[:],
                            in_offset=bass.IndirectOffsetOnAxis(
                                ap=srow[:, t:t + 1], axis=0))
                        oh = rot2.tile([P, P], f32, tag="oh2")
                        nc.vector.tensor_scalar(out=oh[:], in0=iota,
                                                scalar1=dloc[:, t:t + 1],
                                                scalar2=None, op0=ALU.is_equal)
                        msg = rot2.tile([P, F + 1], f32, tag="msg2")
                        nc.vector.tensor_tensor(out=msg[:, 0:F],
                                                in0=g_all[:, t, :],
                                                in1=xg[:], op=ALU.mult)
                        nc.vector.tensor_copy(out=msg[:, F:F + 1],
                                              in_=epair[:, t:t + 1])
                        nc.tensor.matmul(out=x1ps[:], lhsT=oh[:], rhs=msg[:],
                                         start=(j == 0), stop=(j == T_blk - 1))
                    nc.scalar.copy(out=X1sb[:, b, :], in_=x1ps[:])

                molps = psm.tile([P, 1], f32, tag="molps")
                for b in range(NB):
                    trp = rps1.tile([F, P], f32, tag="trp")
                    nc.tensor.transpose(out=trp[:], in_=X1sb[:, b, 0:F],
                                        identity=ident[:])
                    xT = rf2.tile([F, P], f32, tag="xT2")
                    nc.scalar.copy(out=xT[:], in_=trp[:])
                    hps = rps2.tile([P, F], f32, tag="hps")
                    nc.tensor.matmul(out=hps[:], lhsT=xT[:], rhs=w11,
                                     start=True, stop=True)
                    sw = rf2.tile([P, F], f32, tag="sw2")
                    nc.scalar.activation(out=sw[:], in_=hps[:], func=ACT.Silu)
                    gtp = rps1.tile([F, P], f32, tag="trp")
                    nc.tensor.transpose(out=gtp[:], in_=sw[:],
                                        identity=ident[:])
                    gT = rf2.tile([F, P], f32, tag="gT2")
                    nc.scalar.copy(out=gT[:], in_=gtp[:])
                    dps = rps2.tile([P, F], f32, tag="hps")
                    nc.tensor.matmul(out=dps[:], lhsT=gT[:], rhs=w21,
                                     start=True, stop=True)
                    x0b = rf2.tile([P, F], f32, tag="x0b")
                    nc.vector.tensor_tensor(out=x0b[:], in0=X1sb[:, b, 0:F],
                                            in1=dps[:], op=ALU.add)
                    tmp2 = rf2.tile([P, F], f32, tag="tmp2")
                    nc.vector.tensor_tensor(out=tmp2[:], in0=x0b[:],
                                            in1=woutr, op=ALU.mult)
                    ea = rf2.tile([P, 1], f32, tag="ea")
                    nc.vector.tensor_reduce(out=ea[:], in_=tmp2[:],
                                            axis=mybir.AxisListType.X,
                                            op=ALU.add)
                    nc.vector.tensor_tensor(out=ea[:], in0=ea[:],
                                            in1=bout_t[:, b:b + 1],
                                            op=ALU.add)
                    nc.vector.tensor_tensor(out=ea[:], in0=ea[:],
                                            in1=X1sb[:, b, F:F + 1],
                                            op=ALU.add)
                    nc.vector.tensor_tensor(out=ea[:], in0=ea[:],
                                            in1=amask_t[:, b:b + 1],
                                            op=ALU.mult)
                    ohm = rf2.tile([P, P], f32, tag="ohm")
                    nc.vector.tensor_scalar(out=ohm[:], in0=iota,
                                            scalar1=segloc_t[:, b:b + 1],
                                            scalar2=None, op0=ALU.is_equal)
                    nc.tensor.matmul(out=molps[:], lhsT=ohm[:], rhs=ea[:],
                                     start=(b == 0), stop=(b == NB - 1))
                mol = p2.tile([P, 1], f32, tag="mol")
                nc.vector.tensor_copy(out=mol[:], in_=molps[:])
                nc.sync.dma_start(d_out[:, :], mol[:])
    return nc


# --------------------------------------------------------------------------
# cached PJRT dispatcher (jit + shard_map built once per shape)
# --------------------------------------------------------------------------
class _Runner:
    def __init__(self, nc):
        import jax
        import numpy as _np
        from jax.sharding import Mesh, PartitionSpec
        try:
            from jax import shard_map
            def _shard_map(f, mesh, in_specs, out_specs):
                return shard_map(f, mesh=mesh, in_specs=in_specs,
                                 out_specs=out_specs, check_vma=False)
        except ImportError:
            from jax.experimental.shard_map import shard_map
            def _shard_map(f, mesh, in_specs, out_specs):
                return shard_map(f, mesh=mesh, in_specs=in_specs,
                                 out_specs=out_specs, check_rep=False)
        import concourse.mybir as mybir
        from concourse import bass2jax

        bass2jax.install_neuronx_cc_hook()
        self.nc = nc
        partition_name = (nc.partition_id_tensor.name
                          if nc.partition_id_tensor else None)
        in_names, out_names, out_avals, zero_shapes = [], [], [], []
        for alloc in nc.m.functions[0].allocations:
            if not isinstance(alloc, mybir.MemoryLocationSet):
                continue
            name = alloc.memorylocations[0].name
            if alloc.kind == "ExternalInput":
                if name != partition_name:
                    in_names.append(name)
            elif alloc.kind == "ExternalOutput":
                out_names.append(name)
                shape = tuple(alloc.tensor_shape)
                dtype = mybir.dt.np(alloc.dtype)
                out_avals.append(jax.core.ShapedArray(shape, dtype))
                zero_shapes.append((shape, dtype))
        self.in_names = in_names
        self.out_names = out_names
        self.zero_shapes = zero_shapes
        n_params = len(in_names)
        n_outs = len(out_names)
        all_in_names = in_names + out_names + (
            [partition_name] if partition_name else [])
        donate = tuple(range(n_params, n_params + n_outs))

        def _body(*args):
            operands = list(args)
            if partition_name is not None:
                operands.append(bass2jax.partition_id_tensor())
            outs = bass2jax._bass_exec_p.bind(
                *operands, out_avals=tuple(out_avals),
                in_names=tuple(all_in_names), out_names=tuple(out_names),
                lowering_input_output_aliases=(),
                sim_require_finite=True, sim_require_nnan=True, nc=nc)
            return tuple(outs)

        self.sharding = _get_sharding()
        mesh = self.sharding.mesh
        in_specs = (PartitionSpec("core"),) * (n_params + n_outs)
        out_specs = (PartitionSpec("core"),) * n_outs
        self.fn = jax.jit(_shard_map(_body, mesh, in_specs, out_specs),
                          donate_argnums=donate, keep_unused=True)
        self._jax = jax

    def put(self, arr):
        """Async transfer of one [NCORES*rows, ...] array to the devices."""
        return self._jax.device_put(arr, self.sharding)

    def __call__(self, arrays):
        zs = [np.zeros((NCORES * s[0], *s[1:]), d)
              for (s, d) in self.zero_shapes]
        outs = self.fn(*[arrays[n] for n in self.in_names], *zs)
        return {n: np.asarray(outs[i]) for i, n in enumerate(self.out_names)}


def _get_runner(T, T_blk):
    key = (T, T_blk)
    if key not in _CACHE:
        nc = _build(T, T_blk)
        nc.finalize()
        _CACHE[key] = _Runner(nc)
    return _CACHE[key]


def kernel(**inputs):
    """Retry wrapper: the axon terminal occasionally throws
    NRT_EXEC_UNIT_UNRECOVERABLE or returns corrupted (NaN) results; both
    recover on a fresh attempt."""
    import time
    out = None
    for attempt in range(5):
        try:
            out = _kernel_once(**inputs)
            if not np.isnan(out).any():
                return out
        except Exception:
            if attempt == 4:
                raise
        # corruption windows last several seconds; back off progressively
        time.sleep(1.0 * (attempt + 1))
    return out


def _kernel_once(**inputs):
    import jax
    batch_mask = np.asarray(inputs["batch_mask"], np.float32)
    an = np.asarray(inputs["atomic_numbers"]).astype(np.int32)
    sh = _get_sharding()

    # the small tensors don't depend on the edge layout: ship them first so
    # their transfers run under the whole of stage 1
    small, mol_base = _prep_stage2(
        an, inputs["batch_segments"], inputs["atom_mask"],
        inputs["embed"], inputs["Wr1_0"], inputs["Wr2_0"], inputs["W1_0"],
        inputs["W2_0"], inputs["Wr1_1"], inputs["W1_1"], inputs["W2_1"],
        inputs["w_out"], inputs["b_out"])
    staged = {n: jax.device_put(a, sh) for n, a in small.items()}

    def stage_cb(T, T_blk, name, arr):
        staged[name] = jax.device_put(arr, sh)

    T, T_blk = _prep_stage1(
        inputs["positions"], inputs["dst_idx"], inputs["src_idx"],
        an, stage_cb)
    runner = _get_runner(T, T_blk)
    res = runner(staged)

    w = res["out"].reshape(NCORES, P)
    out = np.zeros((B,), dtype=np.float32)
    for c in range(NCORES):
        lo = int(mol_base[c])
        hi = min(lo + P, B)
        out[lo:hi] += w[c, :hi - lo]
    return out * batch_mask


def profile_exec_ns(**inputs):
    raise RuntimeError("NTFF tracing unavailable under this axon client; "
                       "wall-clock is the metric")


# revision 43
# speedup vs baseline: 1.0719x; 1.0719x over previous
"""Bass/Trainium2 kernel for nn_EF_42511586295882 (GNN message passing).

Math reduction proven against reference: only the l=0 spherical channel of
iteration 0 reaches the output (refinement mixes features, never l-channels,
and only x[:, 0, :] feeds iteration 1 / readout).  The whole computation is:

  rad[e,k]  = T_k(2*exp(-r)-1) * cut(r)                        (E,16)
  msg0[e,f] = (rad @ (0.282095*Wr1_0 + Wr2_0))[e,f] * embed[z[src_e], f]
  X0[a,f]   = sum_{e: dst=a} msg0[e,f]
  x0        = X0 + (h0 * silu(h0)) @ W2_0,   h0 = X0 @ W1_0
  msg1[e,f] = (rad @ Wr1_1)[e,f] * x0[src_e, f]
  X1[a,f]   = sum_{e: dst=a} msg1[e,f]
  x0b       = X1 + silu(X1 @ W1_1) @ W2_1
  e_atom    = x0b @ w_out + b_out[z] + zbl_pair_sum[a]
  e_mol     = segment_sum(e_atom * atom_mask, batch_segments)

This implementation is optimized for END-TO-END wall time of kernel():
host numpy prep + PJRT/axon transfer + device execution.  Key choices:

  * ONE device launch (no host round-trip between the two message passes):
    x0 is exchanged on-device with an 8-core DRAM AllGather, and x0[src] /
    embed[z_src] are fetched per 128-edge tile with indirect (indexed) DMA.
  * Only 7B/edge is shipped (r f16, dst-slot u8, z_src u8, z_dst u8,
    src-row u16); all O(E) and O(E*F) math, including the ZBL pair energy,
    happens on device.  Host prep is ~15ms of numpy, overlapped with the
    transfers (layout-independent tensors ship first).
  * Edges only need to be GROUPED by 128-atom destination block, not fully
    sorted: argsort on a uint8 block key is ~20x faster than on dst.
  * The jax.jit(shard_map(...)) dispatcher is built once and cached
    (rebuilding it per call costs ~0.15s extra), and the edge planes are
    device_put asynchronously as soon as each is computed so transfers
    overlap the remaining host prep.  The per-launch round-trip floor of
    this axon stack is ~75ms; everything else is hidden behind it.

Sharding: core k owns atoms [2048k, 2048(k+1)) and all edges into them,
grouped in 16 aligned 128-atom blocks.  Scatter = one-hot matmul into a
per-block PSUM accumulator.
"""

import math
import numpy as np

P = 128
N = 16384
E = 262144
B = 512
F = 32
K = 16
NZ = 119
NCORES = 8
AC = N // NCORES          # atoms per core
NB = AC // P              # 128-atom blocks per core (16)
CUTOFF = 6.0
KE = 14.399645
ZBL_C = [0.18175, 0.50986, 0.28022, 0.02817]
ZBL_D = [3.19980, 0.94229, 0.40290, 0.20162]
A_PRE = 0.8854 * 0.529177

_CACHE = {}
_SHARDING = None


def _get_sharding():
    """Mesh/sharding is fixed (8 cores, rows split) independent of T, so
    transfers can start before the edge layout — and thus the runner — is
    known."""
    global _SHARDING
    if _SHARDING is None:
        import jax
        from jax.sharding import Mesh, PartitionSpec, NamedSharding
        mesh = Mesh(np.asarray(jax.devices()[:NCORES]), ("core",))
        _SHARDING = NamedSharding(mesh, PartitionSpec("core"))
    return _SHARDING


# --------------------------------------------------------------------------
# host prep: group edges by dst 128-block, lay them out in (core, 128-slot,
# tile) grid, compute r + ZBL pair energies, emit [NCORES*P, ...] arrays.
# Split in two stages so the big edge planes can start transferring to the
# devices (async device_put) while stage 2 runs.
# --------------------------------------------------------------------------
_ARANGE_E = None


_SROW_LUT = None


def _prep_stage1(positions, dst_idx, src_idx, an, stage_cb):
    """Build the per-edge planes.  ``stage_cb(T, T_blk, name, arr)`` is
    called as soon as each plane is final so its device transfer can start
    while the remaining planes are still being computed.  The r chain runs
    on the main thread concurrently with the integer blob in a worker."""
    global _ARANGE_E, _SROW_LUT
    import concurrent.futures as _cf
    pos = np.asarray(positions, dtype=np.float32)
    dst = np.asarray(dst_idx).astype(np.int32)
    src = np.asarray(src_idx).astype(np.int32)

    # group by destination 128-atom block (order within a block is free)
    order = np.argsort((dst >> 7).astype(np.uint8), kind="stable")
    dsts = np.take(dst, order)
    srcs = np.take(src, order)

    gb = dsts >> 7                       # global 128-atom block id (0..127)
    cnt = np.bincount(gb, minlength=N // P)
    T_blk = int(math.ceil(cnt.max() / P))
    T = NB * T_blk

    starts = np.zeros(N // P, dtype=np.int32)
    np.cumsum(cnt[:-1], out=starts[1:], dtype=np.int32)
    if _ARANGE_E is None or len(_ARANGE_E) != len(dsts):
        _ARANGE_E = np.arange(len(dsts), dtype=np.int32)
    k = _ARANGE_E - np.take(starts, gb)
    # flat position in concat [NCORES*P, T] via two small LUTs:
    # flat = (gb>>4)*(P*T) + (gb&15)*T_blk  +  (k&127)*T + (k>>7)
    lut_g = ((np.arange(N // P, dtype=np.int32) >> 4) * (P * T)
             + (np.arange(N // P, dtype=np.int32) & 15) * T_blk)
    kmax = int(cnt.max())
    lut_k = ((np.arange(kmax, dtype=np.int32) & 127) * T
             + (np.arange(kmax, dtype=np.int32) >> 7))
    flat = np.take(lut_g, gb)
    flat += np.take(lut_k, k)

    NPALL = NCORES * P

    def int_planes():
        # one u8 blob [NPALL, 5T]: dloc | zsrc | zdst | srow(u16)
        blob = np.zeros((NPALL, 5 * T), dtype=np.uint8)
        dlo = np.zeros((NPALL * T,), dtype=np.uint8)
        dlo[flat] = (dsts & 127).astype(np.uint8)
        blob[:, 0:T] = dlo.reshape(NPALL, T)
        an8 = an.astype(np.uint8)
        zsr = np.zeros((NPALL * T,), dtype=np.uint8)
        zsr[flat] = np.take(an8, srcs)
        blob[:, T:2 * T] = zsr.reshape(NPALL, T)
        zds = np.zeros((NPALL * T,), dtype=np.uint8)
        zds[flat] = np.take(an8, dsts)
        blob[:, 2 * T:3 * T] = zds.reshape(NPALL, T)
        # srow: row of atom a in the allgathered x0 table [N, F]:
        # R(a) = (a//2048)*2048 + (a%128)*16 + (a%2048)//128  (LUT over a)
        global _SROW_LUT
        if _SROW_LUT is None:
            a_all = np.arange(N, dtype=np.int32)
            _SROW_LUT = (((a_all >> 11) << 11) + ((a_all & 127) << 4)
                         + ((a_all & 2047) >> 7)).astype(np.uint16)
        srow = np.zeros((NPALL * T,), dtype=np.uint16)
        srow[flat] = np.take(_SROW_LUT, srcs)
        blob[:, 3 * T:5 * T] = srow.view(np.uint8).reshape(NPALL, 2 * T)
        return blob

    with _cf.ThreadPoolExecutor(max_workers=1) as ex:
        bi_fut = ex.submit(int_planes)

        # per-edge r via contiguous per-column ops (main thread)
        px, py, pz = pos[:, 0].copy(), pos[:, 1].copy(), pos[:, 2].copy()
        dx = np.take(px, srcs)
        dx -= np.take(px, dsts)
        dy = np.take(py, srcs)
        dy -= np.take(py, dsts)
        dz = np.take(pz, srcs)
        dz -= np.take(pz, dsts)
        np.multiply(dx, dx, out=dx)
        np.multiply(dy, dy, out=dy)
        np.multiply(dz, dz, out=dz)
        dx += dy
        dx += dz
        dx += 1e-10
        r = np.sqrt(dx, out=dx)
        np.maximum(r, 1e-4, out=r)
        # f16 r: rel err ~5e-4 on r -> 1.7e-4 on e_mol (gate 2e-2), halves the plane
        rpl = np.full((NPALL * T,), 1000.0, dtype=np.float16)  # padded: cut=0
        rpl[flat] = r.astype(np.float16)

        blob = bi_fut.result()
    stage_cb(T, T_blk, "blob_i", blob)
    stage_cb(T, T_blk, "r_t", rpl.reshape(NPALL, T))

    return T, T_blk


def _prep_stage2(an, batch_segments, atom_mask, embed,
                 Wr1_0, Wr2_0, W1_0, W2_0, Wr1_1, W1_1, W2_1, w_out, b_out):
    seg = np.asarray(batch_segments).astype(np.int64)

    # ---- per-atom planes [NCORES*P, NB]: atom a=(c,b,p) -> row c*128+p, col b
    def atom_plane(v):
        return v.reshape(NCORES, NB, P).transpose(0, 2, 1)

    mol_base = seg.reshape(NCORES, AC)[:, 0]
    segloc = (seg - np.repeat(mol_base, AC)).astype(np.float32)
    assert segloc.max() < P, "molecule window exceeds 128 per core"

    # iota | w_out | b_out[z] | segloc | atom_mask in one [P, 208] array
    ioa = np.empty((NCORES, P, P + F + 3 * NB), dtype=np.float32)
    ioa[:, :, 0:P] = np.arange(P, dtype=np.float32)[None, None, :]
    ioa[:, :, P:P + F] = np.asarray(w_out, np.float32)[None, None, :]
    o = P + F
    ioa[:, :, o:o + NB] = atom_plane(np.take(np.asarray(b_out, np.float32), an))
    ioa[:, :, o + NB:o + 2 * NB] = atom_plane(segloc)
    ioa[:, :, o + 2 * NB:o + 3 * NB] = atom_plane(np.asarray(atom_mask, np.float32))

    # ---- replicated small tensors ----
    gcW = 0.282095 * np.asarray(Wr1_0, np.float32) + np.asarray(Wr2_0, np.float32)
    wcatS = np.empty((K, 2 * F), dtype=np.float32)
    wcatS[:, 0:F] = gcW
    wcatS[:, F:2 * F] = np.asarray(Wr1_1, np.float32)

    wpack = np.empty((F, 4 * F), dtype=np.float32)
    wpack[:, 0 * F:1 * F] = np.asarray(W1_0, np.float32)
    wpack[:, 1 * F:2 * F] = np.asarray(W2_0, np.float32)
    wpack[:, 2 * F:3 * F] = np.asarray(W1_1, np.float32)
    wpack[:, 3 * F:4 * F] = np.asarray(W2_1, np.float32)

    embP = np.zeros((P, F), dtype=np.float32)
    embP[:NZ] = np.asarray(embed, dtype=np.float32)

    def rep(a):            # replicate a per-core tensor 8x along axis 0
        return np.ascontiguousarray(
            np.broadcast_to(a[None], (NCORES, *a.shape))).reshape(
                NCORES * a.shape[0], *a.shape[1:])

    arrays = {
        "ioa": ioa.reshape(NCORES * P, P + F + 3 * NB),
        "wcatS": rep(wcatS),
        "wpack": rep(wpack),
        "embP": rep(embP),
    }
    return arrays, mol_base


# --------------------------------------------------------------------------
# device kernel: one NEFF doing pass1 -> refine0 -> AllGather -> pass2 ->
# refine1 -> readout
# --------------------------------------------------------------------------
def _build(T, T_blk):
    import concourse.bacc as bacc
    import concourse.bass as bass
    import concourse.mybir as mybir
    import concourse.tile as tile
    from concourse.masks import make_identity

    f32 = mybir.dt.float32
    f16 = mybir.dt.float16
    i32 = mybir.dt.int32
    u8 = mybir.dt.uint8
    u16 = mybir.dt.uint16
    ALU = mybir.AluOpType
    ACT = mybir.ActivationFunctionType

    nc = bacc.Bacc("TRN2", target_bir_lowering=False, debug=False,
                   num_devices=NCORES)

    d_r = nc.dram_tensor("r_t", [P, T], f16, kind="ExternalInput")
    d_bi = nc.dram_tensor("blob_i", [P, 5 * T], u8, kind="ExternalInput")
    d_wcatS = nc.dram_tensor("wcatS", [K, 2 * F], f32, kind="ExternalInput")
    d_wpack = nc.dram_tensor("wpack", [F, 4 * F], f32, kind="ExternalInput")
    d_ioa = nc.dram_tensor("ioa", [P, P + F + 3 * NB], f32,
                           kind="ExternalInput")
    d_embP = nc.dram_tensor("embP", [P, F], f32, kind="ExternalInput")
    d_out = nc.dram_tensor("out", [P, 1], f32, kind="ExternalOutput")

    with tile.TileContext(nc) as tc:
        with tc.tile_pool(name="const", bufs=1) as cpool, \
             tc.tile_pool(name="persist", bufs=1) as pp, \
             tc.tile_pool(name="dram", bufs=1, space="DRAM") as dpool:

            ident = cpool.tile([P, P], f32, tag="ident")
            make_identity(nc, ident[:])
            ioa = cpool.tile([P, P + F + 3 * NB], f32, tag="ioa")
            nc.sync.dma_start(ioa[:], d_ioa[:, :])
            iota = ioa[:, 0:P]
            woutr = ioa[:, P:P + F]
            _o = P + F
            bout_t = ioa[:, _o:_o + NB]
            segloc_t = ioa[:, _o + NB:_o + 2 * NB]
            amask_t = ioa[:, _o + 2 * NB:_o + 3 * NB]
            wcat = cpool.tile([P, 2 * F], f32, tag="wcat")
            for g in range(4):
                nc.sync.dma_start(wcat[32 * g:32 * g + K, :], d_wcatS[:, :])
            wpack = cpool.tile([F, 4 * F], f32, tag="wpack")
            nc.sync.dma_start(wpack[:], d_wpack[:, :])
            w10 = wpack[:, 0 * F:1 * F]
            w20 = wpack[:, 1 * F:2 * F]
            w11 = wpack[:, 2 * F:3 * F]
            w21 = wpack[:, 3 * F:4 * F]

            dloc8 = pp.tile([P, T], u8, tag="dloc8")
            nc.sync.dma_start(dloc8[:], d_bi[:, 0:T])
            dloc = pp.tile([P, T], f32, tag="dloc")
            nc.vector.tensor_copy(out=dloc[:], in_=dloc8[:])
            zsrc8 = pp.tile([P, T], u8, tag="zsrc8")
            nc.sync.dma_start(zsrc8[:], d_bi[:, T:2 * T])
            zsrc = pp.tile([P, T], i32, tag="zsrc")
            nc.vector.tensor_copy(out=zsrc[:], in_=zsrc8[:])
            srow16 = pp.tile([P, T], u16, tag="srow16")
            nc.sync.dma_start(srow16[:], d_bi[:, 3 * T:5 * T].bitcast(u16))
            srow = pp.tile([P, T], i32, tag="srow")
            nc.vector.tensor_copy(out=srow[:], in_=srow16[:])

            g_all = pp.tile([P, T, F], f32, tag="g_all")
            epair = pp.tile([P, T], f32, tag="epair")
            X0sb = pp.tile([P, NB, F], f32, tag="X0sb")
            x0sb = pp.tile([P, NB, F], f32, tag="x0sb")

            in_b = dpool.tile([P, NB * F], f32)
            x0tab = dpool.tile([N, F], f32)

            # ---------------- pass 1: edge math + scatter ----------------
            with tc.tile_pool(name="p1", bufs=1) as p1, \
                 tc.tile_pool(name="rot", bufs=4) as rot, \
                 tc.tile_pool(name="ps_rt", bufs=2, space="PSUM") as ps_rt, \
                 tc.tile_pool(name="ps_g", bufs=2, space="PSUM") as ps_g, \
                 tc.tile_pool(name="ps_x", bufs=2, space="PSUM") as ps_x:

                r16 = p1.tile([P, T], f16, tag="r16")
                nc.sync.dma_start(r16[:], d_r[:, :])
                r = p1.tile([P, T], f32, tag="r")
                nc.vector.tensor_copy(out=r[:], in_=r16[:])

                # t = 2*exp(-r) - 1 ; t2 = 2*t
                tch = p1.tile([P, T], f32, tag="tch")
                nc.scalar.activation(out=tch[:], in_=r[:], func=ACT.Exp,
                                     scale=-1.0)
                t2 = p1.tile([P, T], f32, tag="t2")
                nc.vector.tensor_scalar(out=t2[:], in0=tch[:], scalar1=4.0,
                                        scalar2=-2.0, op0=ALU.mult, op1=ALU.add)
                nc.vector.tensor_scalar(out=tch[:], in0=tch[:], scalar1=2.0,
                                        scalar2=-1.0, op0=ALU.mult, op1=ALU.add)

                # cut = exp(-u2/(1-u2)), u = min(r/C, 1-1e-6)
                u = p1.tile([P, T], f32, tag="u")
                nc.vector.tensor_scalar(out=u[:], in0=r[:],
                                        scalar1=1.0 / CUTOFF,
                                        scalar2=1.0 - 1e-6,
                                        op0=ALU.mult, op1=ALU.min)
                u2 = p1.tile([P, T], f32, tag="u2")
                nc.vector.tensor_tensor(out=u2[:], in0=u[:], in1=u[:],
                                        op=ALU.mult)
                den = p1.tile([P, T], f32, tag="den")
                nc.vector.tensor_scalar(out=den[:], in0=u2[:], scalar1=-1.0,
                                        scalar2=1.0, op0=ALU.mult, op1=ALU.add)
                nc.vector.reciprocal(out=den[:], in_=den[:])
                frac = p1.tile([P, T], f32, tag="frac")
                nc.vector.tensor_tensor(out=frac[:], in0=u2[:], in1=den[:],
                                        op=ALU.mult)
                cutm = p1.tile([P, T], f32, tag="cutm")
                nc.scalar.activation(out=cutm[:], in_=frac[:], func=ACT.Exp,
                                     scale=-1.0)

                # Chebyshev ladder seeded with cut: rad_k = T_k(t)*cut
                rad = p1.tile([P, T, 2 * K], f32, tag="rad")
                nc.vector.memset(rad[:], 0.0)
                nc.vector.tensor_copy(out=rad[:, :, 0], in_=cutm[:])
                nc.vector.tensor_tensor(out=rad[:, :, 1], in0=tch[:],
                                        in1=cutm[:], op=ALU.mult)
                tmp = p1.tile([P, T], f32, tag="tmp")
                for kk in range(2, K):
                    nc.vector.tensor_tensor(out=tmp[:], in0=t2[:],
                                            in1=rad[:, :, kk - 1], op=ALU.mult)
                    nc.vector.tensor_tensor(out=rad[:, :, kk], in0=tmp[:],
                                            in1=rad[:, :, kk - 2],
                                            op=ALU.subtract)

                # ---- ZBL pair energy (scattered via pass-2 extra column) --
                zdst8 = p1.tile([P, T], u8, tag="zdst8")
                nc.sync.dma_start(zdst8[:], d_bi[:, 2 * T:3 * T])
                zdf = p1.tile([P, T], f32, tag="zdf")
                nc.vector.tensor_copy(out=zdf[:], in_=zdst8[:])
                zsf = p1.tile([P, T], f32, tag="zsf")
                nc.vector.tensor_copy(out=zsf[:], in_=zsrc8[:])
                zz = p1.tile([P, T], f32, tag="zz")
                nc.vector.tensor_tensor(out=zz[:], in0=zdf[:], in1=zsf[:],
                                        op=ALU.mult)
                # zpow = exp(0.23*ln(max(z,1))); z=0 edges have zz=0 anyway
                lnz = p1.tile([P, T], f32, tag="lnz")
                zpd = p1.tile([P, T], f32, tag="zpd")
                nc.vector.tensor_scalar_max(out=zpd[:], in0=zdf[:], scalar1=1.0)
                nc.scalar.activation(out=lnz[:], in_=zpd[:], func=ACT.Ln)
                nc.scalar.activation(out=zpd[:], in_=lnz[:], func=ACT.Exp,
                                     scale=0.23)
                zps = p1.tile([P, T], f32, tag="zps")
                nc.vector.tensor_scalar_max(out=zps[:], in0=zsf[:], scalar1=1.0)
                nc.scalar.activation(out=lnz[:], in_=zps[:], func=ACT.Ln)
                nc.scalar.activation(out=zps[:], in_=lnz[:], func=ACT.Exp,
                                     scale=0.23)
                ra = p1.tile([P, T], f32, tag="ra")
                nc.vector.tensor_tensor(out=ra[:], in0=zpd[:], in1=zps[:],
                                        op=ALU.add)
                nc.vector.tensor_tensor(out=ra[:], in0=ra[:], in1=r[:],
                                        op=ALU.mult)
                nc.vector.tensor_scalar_mul(out=ra[:], in0=ra[:],
                                            scalar1=1.0 / A_PRE)
                phi = p1.tile([P, T], f32, tag="phi")
                ej = p1.tile([P, T], f32, tag="ej")
                for jj in range(4):
                    nc.scalar.activation(out=ej[:], in_=ra[:], func=ACT.Exp,
                                         scale=-ZBL_D[jj])
                    if jj == 0:
                        nc.vector.tensor_scalar_mul(out=phi[:], in0=ej[:],
                                                    scalar1=ZBL_C[jj])
                    else:
                        nc.vector.tensor_scalar_mul(out=ej[:], in0=ej[:],
                                                    scalar1=ZBL_C[jj])
                        nc.vector.tensor_tensor(out=phi[:], in0=phi[:],
                                                in1=ej[:], op=ALU.add)
                rinv = p1.tile([P, T], f32, tag="rinv")
                nc.vector.reciprocal(out=rinv[:], in_=r[:])
                nc.vector.tensor_tensor(out=epair[:], in0=zz[:], in1=phi[:],
                                        op=ALU.mult)
                nc.vector.tensor_tensor(out=epair[:], in0=epair[:],
                                        in1=rinv[:], op=ALU.mult)
                nc.vector.tensor_tensor(out=epair[:], in0=epair[:],
                                        in1=cutm[:], op=ALU.mult)
                nc.vector.tensor_scalar_mul(out=epair[:], in0=epair[:],
                                            scalar1=0.5 * KE)

                for b in range(NB):
                    x0ps = ps_x.tile([P, F], f32, tag="x0ps")
                    for j in range(T_blk):
                        t = b * T_blk + j
                        g4 = t % 4
                        if g4 == 0:
                            radT = ps_rt.tile([P, P], f32, tag="radT")
                            hi = min(4, T - t)
                            nc.tensor.transpose(
                                out=radT[0:32 * hi, :],
                                in_=rad[:, t:t + hi, :],
                                identity=ident[:])
                            radTs = rot.tile([P, P], f32, tag="radTs")
                            nc.scalar.copy(out=radTs[0:32 * hi, :],
                                           in_=radT[0:32 * hi, :])
                        gps = ps_g.tile([P, 2 * F], f32, tag="gps")
                        nc.tensor.matmul(out=gps[:],
                                         lhsT=radTs[32 * g4:32 * g4 + 32, :],
                                         rhs=wcat[32 * g4:32 * g4 + 32, :],
                                         start=True, stop=True,
                                         tile_position=(32 * g4, 0))
                        xs0 = rot.tile([P, F], f32, tag="xs0")
                        nc.gpsimd.indirect_dma_start(
                            out=xs0[:], out_offset=None,
                            in_=d_embP[:, :],
                            in_offset=bass.IndirectOffsetOnAxis(
                                ap=zsrc[:, t:t + 1], axis=0))
                        oh = rot.tile([P, P], f32, tag="oh")
                        nc.vector.tensor_scalar(out=oh[:], in0=iota,
                                                scalar1=dloc[:, t:t + 1],
                                                scalar2=None, op0=ALU.is_equal)
                        msg = rot.tile([P, F], f32, tag="msg")
                        nc.vector.tensor_tensor(out=msg[:], in0=gps[:, 0:F],
                                                in1=xs0[:], op=ALU.mult)
                        nc.scalar.copy(out=g_all[:, t, :], in_=gps[:, F:2 * F])
                        nc.tensor.matmul(out=x0ps[:], lhsT=oh[:], rhs=msg[:],
                                         start=(j == 0), stop=(j == T_blk - 1))
                    nc.scalar.copy(out=X0sb[:, b, :], in_=x0ps[:])

            # ---------------- refinement 0 ----------------
            with tc.tile_pool(name="rf", bufs=2) as rf, \
                 tc.tile_pool(name="rps1", bufs=2, space="PSUM") as rps1, \
                 tc.tile_pool(name="rps2", bufs=2, space="PSUM") as rps2:
                for b in range(NB):
                    trp = rps1.tile([F, P], f32, tag="trp")
                    nc.tensor.transpose(out=trp[:], in_=X0sb[:, b, :],
                                        identity=ident[:])
                    xT = rf.tile([F, P], f32, tag="xT")
                    nc.scalar.copy(out=xT[:], in_=trp[:])
                    hps = rps2.tile([P, F], f32, tag="hps")
                    nc.tensor.matmul(out=hps[:], lhsT=xT[:], rhs=w10,
                                     start=True, stop=True)
                    sw = rf.tile([P, F], f32, tag="sw")
                    nc.scalar.activation(out=sw[:], in_=hps[:], func=ACT.Silu)
                    gate = rf.tile([P, F], f32, tag="gate")
                    nc.vector.tensor_tensor(out=gate[:], in0=hps[:], in1=sw[:],
                                            op=ALU.mult)
                    gtp = rps1.tile([F, P], f32, tag="trp")
                    nc.tensor.transpose(out=gtp[:], in_=gate[:],
                                        identity=ident[:])
                    gT = rf.tile([F, P], f32, tag="gT")
                    nc.scalar.copy(out=gT[:], in_=gtp[:])
                    dps = rps2.tile([P, F], f32, tag="hps")
                    nc.tensor.matmul(out=dps[:], lhsT=gT[:], rhs=w20,
                                     start=True, stop=True)
                    nc.vector.tensor_tensor(out=x0sb[:, b, :],
                                            in0=X0sb[:, b, :], in1=dps[:],
                                            op=ALU.add)

            # ---------------- exchange: AllGather x0 ----------------
            nc.sync.dma_start(in_b[:], x0sb[:])
            nc.gpsimd.collective_compute(
                "AllGather", ALU.bypass,
                replica_groups=[list(range(NCORES))],
                ins=[in_b.opt()], outs=[x0tab.opt()])

            # ---------------- pass 2 + refinement 1 + readout -------------
            with tc.tile_pool(name="p2", bufs=1) as p2, \
                 tc.tile_pool(name="rot2", bufs=4) as rot2, \
                 tc.tile_pool(name="rf2", bufs=2) as rf2, \
                 tc.tile_pool(name="p2ps", bufs=2, space="PSUM") as p2ps, \
                 tc.tile_pool(name="rps1", bufs=2, space="PSUM") as rps1, \
                 tc.tile_pool(name="rps2", bufs=2, space="PSUM") as rps2, \
                 tc.tile_pool(name="psm", bufs=1, space="PSUM") as psm:

                X1sb = p2.tile([P, NB, F + 1], f32, tag="X1sb")
                for b in range(NB):
                    x1ps = p2ps.tile([P, F + 1], f32, tag="x1ps")
                    for j in range(T_blk):
                        t = b * T_blk + j
                        xg = rot2.tile([P, F], f32, tag="xg")
                        nc.gpsimd.indirect_dma_start(
                            out=xg[:], out_offset=None,
                            in_=x0tab[:],
                            in_offset=bass.IndirectOffsetOnAxis(
                                ap=srow[:, t:t + 1], axis=0))
                        oh = rot2.tile([P, P], f32, tag="oh2")
                        nc.vector.tensor_scalar(out=oh[:], in0=iota,
                                                scalar1=dloc[:, t:t + 1],
                                                scalar2=None, op0=ALU.is_equal)
                        msg = rot2.tile([P, F + 1], f32, tag="msg2")
                        nc.vector.tensor_tensor(out=msg[:, 0:F],
                                                in0=g_all[:, t, :],
                                                in1=xg[:], op=ALU.mult)
                        nc.vector.tensor_copy(out=msg[:, F:F + 1],
                                              in_=epair[:, t:t + 1])
                        nc.tensor.matmul(out=x1ps[:], lhsT=oh[:], rhs=msg[:],
                                         start=(j == 0), stop=(j == T_blk - 1))
                    nc.scalar.copy(out=X1sb[:, b, :], in_=x1ps[:])

                molps = psm.tile([P, 1], f32, tag="molps")
                for b in range(NB):
                    trp = rps1.tile([F, P], f32, tag="trp")
                    nc.tensor.transpose(out=trp[:], in_=X1sb[:, b, 0:F],
                                        identity=ident[:])
                    xT = rf2.tile([F, P], f32, tag="xT2")
                    nc.scalar.copy(out=xT[:], in_=trp[:])
                    hps = rps2.tile([P, F], f32, tag="hps")
                    nc.tensor.matmul(out=hps[:], lhsT=xT[:], rhs=w11,
                                     start=True, stop=True)
                    sw = rf2.tile([P, F], f32, tag="sw2")
                    nc.scalar.activation(out=sw[:], in_=hps[:], func=ACT.Silu)
                    gtp = rps1.tile([F, P], f32, tag="trp")
                    nc.tensor.transpose(out=gtp[:], in_=sw[:],
                                        identity=ident[:])
                    gT = rf2.tile([F, P], f32, tag="gT2")
                    nc.scalar.copy(out=gT[:], in_=gtp[:])
                    dps = rps2.tile([P, F], f32, tag="hps")
                    nc.tensor.matmul(out=dps[:], lhsT=gT[:], rhs=w21,
                                     start=True, stop=True)
                    x0b = rf2.tile([P, F], f32, tag="x0b")
                    nc.vector.tensor_tensor(out=x0b[:], in0=X1sb[:, b, 0:F],
                                            in1=dps[:], op=ALU.add)
                    tmp2 = rf2.tile([P, F], f32, tag="tmp2")
                    nc.vector.tensor_tensor(out=tmp2[:], in0=x0b[:],
                                            in1=woutr, op=ALU.mult)
                    ea = rf2.tile([P, 1], f32, tag="ea")
                    nc.vector.tensor_reduce(out=ea[:], in_=tmp2[:],
                                            axis=mybir.AxisListType.X,
                                            op=ALU.add)
                    nc.vector.tensor_tensor(out=ea[:], in0=ea[:],
                                            in1=bout_t[:, b:b + 1],
                                            op=ALU.add)
                    nc.vector.tensor_tensor(out=ea[:], in0=ea[:],
                                            in1=X1sb[:, b, F:F + 1],
                                            op=ALU.add)
                    nc.vector.tensor_tensor(out=ea[:], in0=ea[:],
                                            in1=amask_t[:, b:b + 1],
                                            op=ALU.mult)
                    ohm = rf2.tile([P, P], f32, tag="ohm")
                    nc.vector.tensor_scalar(out=ohm[:], in0=iota,
                                            scalar1=segloc_t[:, b:b + 1],
                                            scalar2=None, op0=ALU.is_equal)
                    nc.tensor.matmul(out=molps[:], lhsT=ohm[:], rhs=ea[:],
                                     start=(b == 0), stop=(b == NB - 1))
                mol = p2.tile([P, 1], f32, tag="mol")
                nc.vector.tensor_copy(out=mol[:], in_=molps[:])
                nc.sync.dma_start(d_out[:, :], mol[:])
    return nc


# --------------------------------------------------------------------------
# cached PJRT dispatcher (jit + shard_map built once per shape)
# --------------------------------------------------------------------------
class _Runner:
    def __init__(self, nc):
        import jax
        import numpy as _np
        from jax.sharding import Mesh, PartitionSpec
        try:
            from jax import shard_map
            def _shard_map(f, mesh, in_specs, out_specs):
                return shard_map(f, mesh=mesh, in_specs=in_specs,
                                 out_specs=out_specs, check_vma=False)
        except ImportError:
            from jax.experimental.shard_map import shard_map
            def _shard_map(f, mesh, in_specs, out_specs):
                return shard_map(f, mesh=mesh, in_specs=in_specs,
                                 out_specs=out_specs, check_rep=False)
        import concourse.mybir as mybir
        from concourse import bass2jax

        bass2jax.install_neuronx_cc_hook()
        self.nc = nc
        partition_name = (nc.partition_id_tensor.name
                          if nc.partition_id_tensor else None)
        in_names, out_names, out_avals, zero_shapes = [], [], [], []
        for alloc in nc.m.functions[0].allocations:
            if not isinstance(alloc, mybir.MemoryLocationSet):
                continue
            name = alloc.memorylocations[0].name
            if alloc.kind == "ExternalInput":
                if name != partition_name:
                    in_names.append(name)
            elif alloc.kind == "ExternalOutput":
                out_names.append(name)
                shape = tuple(alloc.tensor_shape)
                dtype = mybir.dt.np(alloc.dtype)
                out_avals.append(jax.core.ShapedArray(shape, dtype))
                zero_shapes.append((shape, dtype))
        self.in_names = in_names
        self.out_names = out_names
        self.zero_shapes = zero_shapes
        n_params = len(in_names)
        n_outs = len(out_names)
        all_in_names = in_names + out_names + (
            [partition_name] if partition_name else [])
        donate = tuple(range(n_params, n_params + n_outs))

        def _body(*args):
            operands = list(args)
            if partition_name is not None:
                operands.append(bass2jax.partition_id_tensor())
            outs = bass2jax._bass_exec_p.bind(
                *operands, out_avals=tuple(out_avals),
                in_names=tuple(all_in_names), out_names=tuple(out_names),
                lowering_input_output_aliases=(),
                sim_require_finite=True, sim_require_nnan=True, nc=nc)
            return tuple(outs)

        self.sharding = _get_sharding()
        mesh = self.sharding.mesh
        in_specs = (PartitionSpec("core"),) * (n_params + n_outs)
        out_specs = (PartitionSpec("core"),) * n_outs
        self.fn = jax.jit(_shard_map(_body, mesh, in_specs, out_specs),
                          donate_argnums=donate, keep_unused=True)
        self._jax = jax

    def put(self, arr):
        """Async transfer of one [NCORES*rows, ...] array to the devices."""
        return self._jax.device_put(arr, self.sharding)

    def __call__(self, arrays):
        zs = [np.zeros((NCORES * s[0], *s[1:]), d)
              for (s, d) in self.zero_shapes]
        outs = self.fn(*[arrays[n] for n in self.in_names], *zs)
        return {n: np.asarray(outs[i]) for i, n in enumerate(self.out_names)}


def _get_runner(T, T_blk):
    key = (T, T_blk)
    if key not in _CACHE:
        nc = _build(T, T_blk)
        nc.finalize()
        _CACHE[key] = _Runner(nc)
    return _CACHE[key]


def kernel(**inputs):
    """Retry wrapper: the axon terminal occasionally throws
    NRT_EXEC_UNIT_UNRECOVERABLE or returns corrupted (NaN) results; both
    recover on a fresh attempt."""
    import time
    out = None
    for attempt in range(5):
        try:
            out = _kernel_once(**inputs)
            if not np.isnan(out).any():
                return out
        except Exception:
            if attempt == 4:
                raise
        # corruption windows last several seconds; back off progressively
        time.sleep(1.0 * (attempt + 1))
    return out


def _kernel_once(**inputs):
    import jax
    batch_mask = np.asarray(inputs["batch_mask"], np.float32)
    an = np.asarray(inputs["atomic_numbers"]).astype(np.int32)
    sh = _get_sharding()

    # the small tensors don't depend on the edge layout: ship them first so
    # their transfers run under the whole of stage 1
    small, mol_base = _prep_stage2(
        an, inputs["batch_segments"], inputs["atom_mask"],
        inputs["embed"], inputs["Wr1_0"], inputs["Wr2_0"], inputs["W1_0"],
        inputs["W2_0"], inputs["Wr1_1"], inputs["W1_1"], inputs["W2_1"],
        inputs["w_out"], inputs["b_out"])
    staged = {n: jax.device_put(a, sh) for n, a in small.items()}

    def stage_cb(T, T_blk, name, arr):
        staged[name] = jax.device_put(arr, sh)

    T, T_blk = _prep_stage1(
        inputs["positions"], inputs["dst_idx"], inputs["src_idx"],
        an, stage_cb)
    runner = _get_runner(T, T_blk)
    res = runner(staged)

    w = res["out"].reshape(NCORES, P)
    out = np.zeros((B,), dtype=np.float32)
    for c in range(NCORES):
        lo = int(mol_base[c])
        hi = min(lo + P, B)
        out[lo:hi] += w[c, :hi - lo]
    return out * batch_mask


def profile_exec_ns(**inputs):
    raise RuntimeError("NTFF tracing unavailable under this axon client; "
                       "wall-clock is the metric")
